# revision 1
# baseline (speedup 1.0000x reference)
"""MoE routing kernel for Trainium2 (8 NeuronCores, batch-parallel).

Per batch element b (one NeuronCore each):
    pooled = mean_s x[b]; h = tanh(pooled @ rw1 + rb1)
    logits = h @ rw2 + rb2; probs = softmax(logits)
    top-3 of 4 experts, renormalized: w[e] = probs[e]*(probs[e]>min)/(1-min)
    hid_e = gelu_tanh(x[b] @ zw1[e] + zb1[e]); z_e = hid_e @ zw2[e] + zb2[e]
    out[b] = x[b] + sum_e w[e] * z_e

Only the 3 active experts are computed: the router (on device, fp32)
produces the dropped-expert index; three static "slots" then stream
their expert's weights with register-indexed (runtime-offset) DMAs.
Expert MLPs run in fp32r (11-bit mantissa) at full PE rate; weights are
re-laid-out host-side so each dynamic DMA is a clean row-block slice.
"""
import sys

sys.path.insert(0, "/opt/trn_rl_repo")

import numpy as np

import concourse.bacc as bacc
import concourse.bass as bass
import concourse.mybir as mybir
import concourse.tile as tile
import concourse.bass_isa as bass_isa
from concourse.bass_utils import run_bass_kernel_spmd
from concourse.masks import make_identity

S, D, F, E, H = 512, 1024, 4096, 4, 256
K = 3            # active experts (top-3 of 4)
P = 128
TC = S // P      # 4 token chunks
DC = D // P      # 8 d chunks
FC = F // P      # 32 ff chunks
FE = 8           # ff "eighths" for zw1 streaming
FCE = FC // FE   # 4 ff chunks per eighth
NH = D // 512    # 2 output d halves
F32 = mybir.dt.float32
I32 = mybir.dt.int32
F32R = mybir.dt.float32r
GELU = mybir.ActivationFunctionType.Gelu_apprx_tanh


def build_nc():
    nc = bacc.Bacc("TRN2", target_bir_lowering=False, debug=False)

    x_d = nc.dram_tensor("x", [S, D], F32, kind="ExternalInput")
    rw1_d = nc.dram_tensor("rw1", [D, H], F32, kind="ExternalInput")
    rb1_d = nc.dram_tensor("rb1", [H], F32, kind="ExternalInput")
    rw2_d = nc.dram_tensor("rw2", [H, E], F32, kind="ExternalInput")
    rb2_d = nc.dram_tensor("rb2", [E], F32, kind="ExternalInput")
    zw1_d = nc.dram_tensor("zw1r", [E * P, FC, DC, P], F32R, kind="ExternalInput")
    zb1_d = nc.dram_tensor("zb1r", [E * P, FC], F32, kind="ExternalInput")
    zw2_d = nc.dram_tensor("zw2r", [E * F, D], F32R, kind="ExternalInput")
    zb2_d = nc.dram_tensor("zb2", [E, D], F32, kind="ExternalInput")
    out_d = nc.dram_tensor("out", [S, D], F32, kind="ExternalOutput")

    with tile.TileContext(nc) as tc:
        with (
            tc.tile_pool(name="const", bufs=1) as const,
            tc.tile_pool(name="xb", bufs=1) as xb,
            tc.tile_pool(name="wstream", bufs=1) as wstream,
            tc.tile_pool(name="ps", bufs=8, space="PSUM") as ps,
        ):
            ident = const.tile([P, P], F32, name="ident")
            make_identity(nc, ident)
            onesb = const.tile([P, 1], mybir.dt.bfloat16, name="onesb")
            nc.vector.memset(onesb, 1.0)

            # x as 4 quarter DMAs alternating HWDGE rings
            x_all = xb.tile([P, TC, D], F32, name="x_all")
            xv = x_d.rearrange("(t p) d -> p t d", p=P)
            for t in range(TC):
                eng = nc.sync if t % 2 == 0 else nc.scalar
                eng.dma_start(out=x_all[:, t:t + 1, :], in_=xv[:, t:t + 1, :])
            x_sb = [x_all[:, t, :] for t in range(TC)]

            # small router weights on the scalar ring
            rw1_sb = const.tile([P, DC, H], F32, name="rw1_sb")
            nc.scalar.dma_start(out=rw1_sb, in_=rw1_d.rearrange("(c p) h -> p c h", p=P))
            rb1_row = const.tile([1, H], F32, name="rb1_row")
            nc.scalar.dma_start(out=rb1_row, in_=rb1_d.rearrange("(o h) -> o h", o=1))
            rw2T_sb = const.tile([1, E, H], F32, name="rw2T_sb")
            nc.scalar.dma_start(out=rw2T_sb, in_=rw2_d.rearrange("(o h) e -> o e h", o=1))
            rb2_sb = const.tile([1, E], F32, name="rb2_sb")
            nc.scalar.dma_start(out=rb2_sb, in_=rb2_d.rearrange("(o e) -> o e", o=1))
            zb2_sb = const.tile([1, E, D], F32, name="zb2_sb")
            nc.scalar.dma_start(out=zb2_sb, in_=zb2_d.rearrange("(o e) d -> o e d", o=1))

            # bf16 copy of x for the fast (selection-only) router path
            x_bf = xb.tile([P, TC, D], mybir.dt.bfloat16, name="x_bf")
            for t in range(TC):
                nc.vector.tensor_copy(out=x_bf[:, t, :], in_=x_sb[t])

            # ---------- FAST router (bf16): picks the dropped expert ----------
            # pooled_row[1, D] = sum_t ones.T @ x_bf[t]  (1/S folded in later)
            prow_ps = []
            for nh in range(NH):
                pr = ps.tile([1, 512], F32, name=f"prow{nh}", tag="ps")
                for t in range(TC):
                    nc.tensor.matmul(pr, onesb, x_bf[:, t, nh * 512:(nh + 1) * 512],
                                     start=(t == 0), stop=(t == TC - 1))
                prow_ps.append(pr)
            pooled_row = const.tile([1, D], F32, name="pooled_row")
            for nh in range(NH):
                nc.vector.tensor_copy(out=pooled_row[:, nh * 512:(nh + 1) * 512],
                                      in_=prow_ps[nh])
            pooled_dram = nc.dram_tensor("pooled_bounce", [1, D], F32)
            nc.sync.dma_start(out=pooled_dram[:, :], in_=pooled_row)
            pooled3 = const.tile([P, 1, DC], F32, name="pooled3")
            nc.sync.dma_start(out=pooled3,
                              in_=pooled_dram.rearrange("o (c p) -> p o c", p=P))
            pooled = pooled3[:, 0, :]

            # ---------- transposes t0/t1 (PE busy while DVE runs the chain) ----------
            xT = []
            for dc in range(DC):
                xtd = xb.tile([P, S], F32R, name=f"xT{dc}")
                xT.append(xtd)

            def emit_transposes(trange):
                for t in trange:
                    for dc in range(DC):
                        ptr = ps.tile([P, P], F32, name=f"ptr{t}_{dc}", tag="ps")
                        nc.tensor.transpose(ptr, x_sb[t][:, dc * P:(dc + 1) * P], ident)
                        nc.vector.tensor_copy(out=xT[dc][:, t * P:(t + 1) * P], in_=ptr)

            emit_transposes([0, 1])

            # fast h: hacc[p,h] = sum_dc rw1[p,dc,h]*pooled[p,dc];
            # cross-partition sum on gpsimd, then row-form tanh + logits
            hacc = const.tile([P, H], F32, name="hacc")
            nc.vector.tensor_scalar(out=hacc, in0=rw1_sb[:, 0, :],
                                    scalar1=pooled[:, 0:1], scalar2=None,
                                    op0=mybir.AluOpType.mult)
            for dc in range(1, DC):
                nc.vector.scalar_tensor_tensor(out=hacc, in0=rw1_sb[:, dc, :],
                                               scalar=pooled[:, dc:dc + 1], in1=hacc,
                                               op0=mybir.AluOpType.mult,
                                               op1=mybir.AluOpType.add)
            onesf = const.tile([P, 1], F32, name="onesf")
            nc.vector.memset(onesf, 1.0)
            ph = ps.tile([1, H], F32, name="ph", tag="ps")
            nc.tensor.matmul(ph, onesf, hacc, start=True, stop=True)
            hrow_pre = const.tile([1, H], F32, name="hrow_pre")
            nc.vector.scalar_tensor_tensor(out=hrow_pre, in0=ph,
                                           scalar=1.0 / S,
                                           in1=rb1_row, op0=mybir.AluOpType.mult,
                                           op1=mybir.AluOpType.add)
            h_row = const.tile([1, H], F32, name="h_row")
            nc.scalar.activation(out=h_row, in_=hrow_pre,
                                 func=mybir.ActivationFunctionType.Tanh)

            emit_transposes([2])

            logits = const.tile([1, E], F32, name="logits")
            lscr = const.tile([1, H], F32, name="lscr")
            lsum = const.tile([1, E], F32, name="lsum")
            for e in range(E):
                nc.vector.tensor_mul(lscr, h_row, rw2T_sb[:, e, :])
                nc.vector.tensor_reduce(out=lsum[:, e:e + 1], in_=lscr,
                                        axis=mybir.AxisListType.X,
                                        op=mybir.AluOpType.add)
            nc.vector.tensor_add(logits, lsum, rb2_sb)

            # dropped expert straight from logits (argmin; softmax is monotone)
            lmin = const.tile([1, 1], F32, name="lmin")
            nc.vector.tensor_reduce(out=lmin, in_=logits, axis=mybir.AxisListType.X,
                                    op=mybir.AluOpType.min)
            iota4 = const.tile([1, E], F32, name="iota4")
            for e in range(E):
                nc.vector.memset(iota4[:, e:e + 1], float(e))
            lemask = const.tile([1, E], F32, name="lemask")
            nc.vector.tensor_scalar(out=lemask, in0=logits, scalar1=lmin, scalar2=None,
                                    op0=mybir.AluOpType.is_le)
            emul = const.tile([1, E], F32, name="emul")
            nc.vector.tensor_mul(emul, iota4, lemask)
            dminf = const.tile([1, 1], F32, name="dminf")
            nc.vector.tensor_reduce(out=dminf, in_=emul, axis=mybir.AxisListType.X,
                                    op=mybir.AluOpType.add)
            iota3 = const.tile([1, K], F32, name="iota3")
            for k in range(K):
                nc.vector.memset(iota3[:, k:k + 1], float(k))
            gemask = const.tile([1, K], F32, name="gemask")
            nc.vector.tensor_scalar(out=gemask, in0=iota3, scalar1=dminf, scalar2=None,
                                    op0=mybir.AluOpType.is_ge)
            ekf = const.tile([1, K], F32, name="ekf")
            nc.vector.tensor_add(ekf, iota3, gemask)
            ekP_f = const.tile([1, K], F32, name="ekP_f")
            nc.vector.tensor_scalar(out=ekP_f, in0=ekf, scalar1=float(P), scalar2=None,
                                    op0=mybir.AluOpType.mult)
            ekP_i = const.tile([1, K], I32, name="ekP_i")
            nc.vector.tensor_copy(out=ekP_i, in_=ekP_f)
            ekF_f = const.tile([1, K], F32, name="ekF_f")
            nc.vector.tensor_scalar(out=ekF_f, in0=ekf, scalar1=float(F), scalar2=None,
                                    op0=mybir.AluOpType.mult)
            ekF_i = const.tile([1, K], I32, name="ekF_i")
            nc.vector.tensor_copy(out=ekF_i, in_=ekF_f)

            emit_transposes([3])

            hid = xb.tile([P, FC, S], F32R, name="hid")
            # accumulate in place over x_all (x is dead after the transposes)
            zacc = [x_all[:, t, :] for t in range(TC)]
            wbc3 = const.tile([P, 1, K], F32, name="wbc3")
            wbc = wbc3[:, 0, :]

            # ---------- ACCURATE router chain (for combine weights) ----------
            # emitted as closures, interleaved into slot-0 GEMM1 below so the
            # PE never stalls on it; wc only gates the first STT eviction.
            pooledA = const.tile([P, DC], F32, name="pooledA")
            haccA = const.tile([P, H], F32, name="haccA")
            hrow_preA = const.tile([1, H], F32, name="hrow_preA")
            h_rowA = const.tile([1, H], F32, name="h_rowA")
            logitsA = const.tile([1, E], F32, name="logitsA")
            lsumA_box = [None]

            def emit_accA():
                # accurate pooled from xT (rounded x, free-dim reduce on DVE)
                for dc in range(DC):
                    nc.vector.tensor_reduce(out=pooledA[:, dc:dc + 1],
                                            in_=xT[dc].bitcast(F32),
                                            axis=mybir.AxisListType.X,
                                            op=mybir.AluOpType.add)
                nc.vector.tensor_scalar(out=haccA, in0=rw1_sb[:, 0, :],
                                        scalar1=pooledA[:, 0:1], scalar2=None,
                                        op0=mybir.AluOpType.mult)
                for dc in range(1, DC):
                    nc.vector.scalar_tensor_tensor(out=haccA, in0=rw1_sb[:, dc, :],
                                                   scalar=pooledA[:, dc:dc + 1],
                                                   in1=haccA,
                                                   op0=mybir.AluOpType.mult,
                                                   op1=mybir.AluOpType.add)
                hsumA = const.tile([P, H], F32, name="hsumA")
                nc.gpsimd.partition_all_reduce(hsumA, haccA, channels=P,
                                               reduce_op=bass_isa.ReduceOp.add)
                nc.vector.scalar_tensor_tensor(out=hrow_preA, in0=hsumA[0:1, :],
                                               scalar=1.0 / S, in1=rb1_row,
                                               op0=mybir.AluOpType.mult,
                                               op1=mybir.AluOpType.add)

            def emit_accB():
                nc.scalar.activation(out=h_rowA, in_=hrow_preA,
                                     func=mybir.ActivationFunctionType.Tanh)
                lscrA = const.tile([1, H], F32, name="lscrA")
                lsumA_box[0] = const.tile([1, E], F32, name="lsumA")
                for e in range(E):
                    nc.vector.tensor_mul(lscrA, h_rowA, rw2T_sb[:, e, :])
                    nc.vector.tensor_reduce(out=lsumA_box[0][:, e:e + 1], in_=lscrA,
                                            axis=mybir.AxisListType.X,
                                            op=mybir.AluOpType.add)

            def emit_accC():
                nc.vector.tensor_add(logitsA, lsumA_box[0], rb2_sb)
                mx = const.tile([1, 1], F32, name="mx")
                nc.vector.tensor_reduce(out=mx, in_=logitsA,
                                        axis=mybir.AxisListType.X,
                                        op=mybir.AluOpType.max)
                sh = const.tile([1, E], F32, name="sh")
                nc.vector.tensor_scalar(out=sh, in0=logitsA, scalar1=mx,
                                        scalar2=None, op0=mybir.AluOpType.subtract)
                ex = const.tile([1, E], F32, name="ex")
                nc.scalar.activation(out=ex, in_=sh,
                                     func=mybir.ActivationFunctionType.Exp)
                sm = const.tile([1, 1], F32, name="sm")
                nc.vector.tensor_reduce(out=sm, in_=ex, axis=mybir.AxisListType.X,
                                        op=mybir.AluOpType.add)
                rs = const.tile([1, 1], F32, name="rs")
                nc.vector.reciprocal(out=rs, in_=sm)
                probs = const.tile([1, E], F32, name="probs")
                nc.vector.tensor_scalar(out=probs, in0=ex, scalar1=rs, scalar2=None,
                                        op0=mybir.AluOpType.mult)
                pmin = const.tile([1, 1], F32, name="pmin")
                nc.vector.tensor_reduce(out=pmin, in_=probs,
                                        axis=mybir.AxisListType.X,
                                        op=mybir.AluOpType.min)
                onec = const.tile([1, 1], F32, name="onec")
                nc.vector.memset(onec, 1.0)
                den = const.tile([1, 1], F32, name="den")
                nc.vector.tensor_sub(den, onec, pmin)
                rden = const.tile([1, 1], F32, name="rden")
                nc.vector.reciprocal(out=rden, in_=den)
                gtmask = const.tile([1, E], F32, name="gtmask")
                nc.vector.tensor_scalar(out=gtmask, in0=probs, scalar1=pmin,
                                        scalar2=None, op0=mybir.AluOpType.is_gt)
                wall = const.tile([1, E], F32, name="wall")
                nc.vector.tensor_mul(wall, probs, gtmask)
                w_sb = const.tile([1, E], F32, name="w_sb")
                nc.vector.tensor_scalar(out=w_sb, in0=wall, scalar1=rden,
                                        scalar2=None, op0=mybir.AluOpType.mult)
                wdiff = const.tile([1, K], F32, name="wdiff")
                nc.vector.tensor_sub(wdiff, w_sb[:, 1:E], w_sb[:, 0:K])
                wstep = const.tile([1, K], F32, name="wstep")
                nc.vector.tensor_mul(wstep, wdiff, gemask)
                wc = const.tile([1, K], F32, name="wc")
                nc.vector.tensor_add(wc, w_sb[:, 0:K], wstep)
                nc.gpsimd.partition_broadcast(wbc3[:, 0, :], wc, channels=P)
                # zb2sum and residual init
                zb2sum = const.tile([1, D], F32, name="zb2sum")
                nc.vector.tensor_scalar(out=zb2sum, in0=zb2_sb[:, 0, :],
                                        scalar1=w_sb[:, 0:1], scalar2=None,
                                        op0=mybir.AluOpType.mult)
                for e in range(1, E):
                    nc.vector.scalar_tensor_tensor(out=zb2sum, in0=zb2_sb[:, e, :],
                                                   scalar=w_sb[:, e:e + 1],
                                                   in1=zb2sum,
                                                   op0=mybir.AluOpType.mult,
                                                   op1=mybir.AluOpType.add)
                zb2b3 = const.tile([P, 1, D], F32, name="zb2b3")
                nc.gpsimd.partition_broadcast(zb2b3[:, 0, :], zb2sum, channels=P)
                for t in range(TC):
                    nc.vector.tensor_add(zacc[t], zacc[t], zb2b3[:, 0, :])

            # ---------- 3 expert slots, runtime-indexed weight streams ----------
            for k in range(K):
                rF_sy = nc.sync.alloc_register(f"rF_sy{k}")
                nc.reg_load(rF_sy, ekF_i[:, k:k + 1])
                ekF_s = nc.sync.snap(rF_sy)
                rP_sc = nc.scalar.alloc_register(f"rP_sc{k}")
                nc.reg_load(rP_sc, ekP_i[:, k:k + 1])
                ekP_c = nc.scalar.snap(rP_sc)

                zb1_sb = wstream.tile([P, FC], F32, name=f"zb1_sb{k}", tag="zb1",
                                      bufs=2)
                nc.scalar.dma_start(out=zb1_sb, in_=zb1_d[bass.ds(ekP_c, P), :])

                # ---------- GEMM1 ----------
                PRE = 5
                zw1q = [None] * FC

                def load_zw1(fc, kk=k, ek=ekP_c):
                    t = wstream.tile([P, DC, P], F32R, name=f"zw1q{kk}_{fc}",
                                     tag="zw1q", bufs=PRE + 1)
                    nc.scalar.dma_start(out=t, in_=zw1_d[bass.ds(ek, P), fc, :, :])
                    zw1q[fc] = t

                for fc in range(PRE):
                    load_zw1(fc)
                for fc in range(FC):
                    if fc + PRE < FC:
                        load_zw1(fc + PRE)
                    p1 = ps.tile([P, S], F32, name=f"p1_{k}_{fc}", tag="ps")
                    for dc in range(DC):
                        nc.tensor.matmul(p1, zw1q[fc][:, dc, :], xT[dc],
                                         start=(dc == 0), stop=(dc == DC - 1))
                    nc.scalar.activation(out=hid[:, fc, :], in_=p1, func=GELU,
                                         bias=zb1_sb[:, fc:fc + 1], scale=1.0)
                    if k == 0:
                        if fc == 22:
                            emit_accA()
                        elif fc == 27:
                            emit_accB()
                if k == 0:
                    emit_accC()

                # ---------- GEMM2 ----------
                p2 = []
                for t in range(TC):
                    for nh in range(NH):
                        p2t = ps.tile([P, 512], F32, name=f"p2_{k}_{t}_{nh}",
                                      tag="ps")
                        p2.append(p2t)
                for fc in range(FC):
                    zw2t = wstream.tile([P, D], F32R, name=f"zw2t{k}_{fc}",
                                        tag="zw2t", bufs=6)
                    nc.sync.dma_start(out=zw2t,
                                      in_=zw2_d[bass.ds(ekF_s + fc * P, P), :])
                    for t in range(TC):
                        for nh in range(NH):
                            nc.tensor.matmul(
                                p2[t * NH + nh],
                                hid[:, fc, t * P:(t + 1) * P],
                                zw2t[:, nh * 512:(nh + 1) * 512],
                                start=(fc == 0), stop=(fc == FC - 1))
                for t in range(TC):
                    for nh in range(NH):
                        sl = slice(nh * 512, (nh + 1) * 512)
                        nc.vector.scalar_tensor_tensor(
                            out=zacc[t][:, sl], in0=p2[t * NH + nh],
                            scalar=wbc[:, k:k + 1], in1=zacc[t][:, sl],
                            op0=mybir.AluOpType.mult, op1=mybir.AluOpType.add)
                        if k == K - 1:
                            eng = nc.scalar if (t * NH + nh) % 2 == 0 else nc.sync
                            eng.dma_start(out=out_d[t * P:(t + 1) * P, sl],
                                          in_=zacc[t][:, sl])

    nc.finalize()
    return nc


_NC_CACHE = None


def _get_nc():
    global _NC_CACHE
    if _NC_CACHE is None:
        _NC_CACHE = build_nc()
    return _NC_CACHE


def kernel(x, rw1, rb1, rw2, rb2, zw1, zb1, zw2, zb2, **run_kwargs):
    x = np.ascontiguousarray(np.asarray(x, dtype=np.float32))
    zw1 = np.asarray(zw1, np.float32)
    zw2 = np.asarray(zw2, np.float32)
    zb1 = np.asarray(zb1, np.float32)
    # relayouts matching the kernel's dynamic row-block slicing
    # zw1r[e*P+p, fc, dc, fw] = zw1[e, dc*P+p, fc*P+fw]
    zw1r = np.ascontiguousarray(
        zw1.reshape(E, DC, P, FC, P).transpose(0, 2, 3, 1, 4).reshape(E * P, FC, DC, P))
    zb1r = np.ascontiguousarray(
        zb1.reshape(E, FC, P).transpose(0, 2, 1).reshape(E * P, FC))
    zw2r = np.ascontiguousarray(zw2.reshape(E * F, D))
    shared = {
        "rw1": np.ascontiguousarray(np.asarray(rw1, np.float32)),
        "rb1": np.ascontiguousarray(np.asarray(rb1, np.float32)),
        "rw2": np.ascontiguousarray(np.asarray(rw2, np.float32)),
        "rb2": np.ascontiguousarray(np.asarray(rb2, np.float32)),
        "zw1r": zw1r,
        "zb1r": zb1r,
        "zw2r": zw2r,
        "zb2": np.ascontiguousarray(np.asarray(zb2, np.float32)),
    }
    B = x.shape[0]
    nc = _get_nc()
    in_maps = [dict(shared, x=x[b]) for b in range(B)]
    res = run_bass_kernel_spmd(nc, in_maps, core_ids=list(range(B)), **run_kwargs)
    out = np.stack([res.results[b]["out"] for b in range(B)], axis=0)
    if run_kwargs:
        kernel.last_results = res
    return out


if __name__ == "__main__":
    rng = np.random.default_rng(0)
    inputs = {
        "x": rng.standard_normal((8, S, D)).astype(np.float32),
        "rw1": (rng.standard_normal((D, H)) / np.sqrt(D)).astype(np.float32),
        "rb1": np.zeros(H, np.float32),
        "rw2": (rng.standard_normal((H, E)) / np.sqrt(H)).astype(np.float32),
        "rb2": np.zeros(E, np.float32),
        "zw1": (rng.standard_normal((E, D, F)) / np.sqrt(D)).astype(np.float32),
        "zb1": np.zeros((E, F), np.float32),
        "zw2": (rng.standard_normal((E, F, D)) / np.sqrt(F)).astype(np.float32),
        "zb2": np.zeros((E, D), np.float32),
    }
    out = kernel(**inputs)
    print("out", out.shape, out.dtype, np.abs(out).max())

